# revision 12
# baseline (speedup 1.0000x reference)
"""AWQ W4A16 linear kernel for Trainium2, 8-core tensor-parallel (column-parallel).

Problem: out[m, o] = sum_k x[m, k] * w[o, k]
         w[o, k] = q[o, k] * s[g(k), o] - z[g(k), o],  GROUP_SIZE=128, g(k)=k//128
         x [4096, 8192] f32, qweight [8192, 4096] int32 (bytes, 2 nibbles each),
         wscales/zeros [64, 8192] f32, out [4096, 8192] f32.

Sharding: out_features (O) split across 8 cores, 1024 each; x replicated.

Math restructure: out = x @ (q*s).T + A @ (-z), where A[m, g] = sum_{k in g} x[m, k].
The zero-point term is exactly a rank-64 matmul, appended to the PE accumulation
chain as a K=64 contraction block; A is precomputed on the host from fp32 x.

Per-core device kernel:
  - host supplies q already unpacked to uint8 in [k, o] layout (qT), and the
    scales broadcast along partitions (srep, fp16)
  - DVE: W[g][128 k, 1024 o] = qT_tile * srep_tile  (one tensor_tensor per group)
  - PE: out.T[o-tile, m] accumulated over 64 k-groups + 1 zero-point block,
    fp16 x fp16 -> f32 PSUM, 8 PSUM banks = 8 o-tiles in flight,
    moving operand = x.T tile [128, 512]
  - DVE copies PSUM->SBUF, DMA out.T to DRAM. Host re-transposes/concats.

Note on waitfix: this walrus build caps every instruction at ONE semaphore
wait (EventSemaphore holds two). Tile emits more; we post-process the BIR
JSON, hoisting excess waits into standalone EventSemaphore instructions
inserted before the over-budget instruction on the same engine.
"""
import sys

for _p in ("/opt/trn_rl_repo", "/root/.axon_site/_ro/trn_rl_repo"):
    if _p not in sys.path:
        sys.path.append(_p)

import json
import numpy as np

import concourse.bass as bass
import concourse.mybir as mybir
import concourse.tile as tile
from concourse.bass_utils import run_bass_kernel_spmd

P = 128          # partitions / group size
M = 4096         # rows of x
K = 8192         # in features
O_TOTAL = 8192   # out features
N_CORES = 8
O = O_TOTAL // N_CORES   # 1024 per core
NOT = O // P             # 8 o-tiles per core
G = K // P               # 64 k-groups
MB = 512                 # moving free dim per matmul
NMG = M // MB            # 8 m-groups
XB = 4                   # k-groups per x-load DMA
dt = mybir.dt


def _split_excess_waits(doc):
    for f in doc.get("functions", []):
        for b in f.get("blocks", []):
            out = []
            for i in b.get("instructions", []):
                si = i.get("sync_info")
                waits = (si or {}).get("on_wait") or []
                if len(waits) > 1:
                    excess, keep = waits[:-1], waits[-1:]
                    for k in range(0, len(excess), 2):
                        out.append({
                            "name": f"{i['name']}-wsplit{k}",
                            "opcode": "EventSemaphore",
                            "engine": i["engine"],
                            "ins": [], "outs": [],
                            "debug": i.get("debug", 0),
                            "sync_info": {"on_wait": excess[k:k + 2],
                                          "on_update": []},
                        })
                    si["on_wait"] = keep
                out.append(i)
            b["instructions"] = out
    return doc


def _thin_pe_updates(doc, every=8):
    """Drop per-MATMUL semaphore increments, keeping one on every Nth matmul,
    and renumber every >=-wait on those sems to the next kept tick. Waits are
    >= so this only over-synchronizes (by at most N-1 matmuls)."""
    import bisect

    all_insts = []
    for f in doc.get("functions", []):
        for b in f.get("blocks", []):
            all_insts.extend(b.get("instructions", []))

    mm_sems = set()
    for i in all_insts:
        if i.get("opcode") == "Matmult":
            for u in (i.get("sync_info") or {}).get("on_update") or []:
                if u.get("update_mode") == "sem-inc":
                    mm_sems.add(u["id"])
    # skip sems with exact-value waits (can't renumber those safely)
    for i in all_insts:
        for w in (i.get("sync_info") or {}).get("on_wait") or []:
            if (w.get("id") in mm_sems and w.get("wait_mode") != "sem-ge-imm"
                    and w.get("wait_value", 0) != 0):
                mm_sems.discard(w["id"])

    for s in mm_sems:
        upd_sites = []
        for i in all_insts:
            for u in (i.get("sync_info") or {}).get("on_update") or []:
                if u.get("update_mode") == "sem-inc" and u.get("id") == s:
                    assert u.get("update_value", 1) == 1
                    upd_sites.append((i, u))
        T = len(upd_sites)
        if T == 0:
            continue
        keep = []
        cnt = 0
        for i, u in upd_sites:
            if i.get("opcode") != "Matmult":
                keep.append(True)
            else:
                cnt += 1
                keep.append(cnt % every == 0)
        keep[-1] = True
        pref = [0]
        for k in keep:
            pref.append(pref[-1] + (1 if k else 0))
        kept_pos = [idx + 1 for idx, k in enumerate(keep) if k]  # 1-based

        def new_wait(v):
            if v <= 0:
                return v
            p = bisect.bisect_left(kept_pos, min(v, T))
            return pref[kept_pos[p]]

        for (i, u), k in zip(upd_sites, keep):
            if not k:
                si = i["sync_info"]
                si["on_update"] = [
                    x for x in si["on_update"]
                    if not (x.get("update_mode") == "sem-inc"
                            and x.get("id") == s)]
        for i in all_insts:
            for w in (i.get("sync_info") or {}).get("on_wait") or []:
                if w.get("id") == s and w.get("wait_mode") == "sem-ge-imm":
                    w["wait_value"] = new_wait(w["wait_value"])
    return doc


def _patch_nc(nc):
    orig = nc.to_json_bytes

    def fixed():
        doc = json.loads(orig())
        doc = _thin_pe_updates(doc)
        doc = _split_excess_waits(doc)
        return json.dumps(doc).encode()

    nc.to_json_bytes = fixed


def build_program():
    nc = bass.Bass()
    xT = nc.declare_dram_parameter("xT", [K, M], dt.float16, isOutput=False)
    qT = nc.declare_dram_parameter("qT", [K, O], dt.uint8, isOutput=False)
    ssh = nc.declare_dram_parameter("ssh", [G, O], dt.float16, isOutput=False)
    zneg = nc.declare_dram_parameter("zneg", [G, O], dt.float16, isOutput=False)
    aT = nc.declare_dram_parameter("aT", [G, M], dt.float16, isOutput=False)
    outT = nc.declare_dram_parameter("outT", [O, M], dt.float32, isOutput=True)

    with tile.TileContext(nc) as tc:
        with (
            tc.tile_pool(name="cst", bufs=1) as cst,
            tc.tile_pool(name="qt", bufs=4) as qtp,
            tc.tile_pool(name="sr", bufs=3) as srp,
            tc.tile_pool(name="w", bufs=1) as wp,
            tc.tile_pool(name="xg", bufs=4) as xp,
            tc.tile_pool(name="at", bufs=2) as atp,
            tc.tile_pool(name="ob", bufs=2) as obp,
            tc.tile_pool(name="ps", bufs=1, space="PSUM") as psp,
        ):
            # --- dequant: W[g] = qT[g] * s_bcast[g], one TT op per group ---
            SC = 4  # scale-broadcast chunk (groups per DMA)
            W = []
            znt = cst.tile([G, O], dt.float16)
            srs = {}
            for g in range(G):
                if g % SC == 0:
                    sr = srp.tile([P, SC, O], dt.float16, tag="sr",
                                  name=f"sr{g // SC}")
                    nc.scalar.dma_start(
                        sr[:], ssh[g:g + SC, :].unsqueeze(0)
                        .broadcast_to([P, SC, O]))
                    srs[g // SC] = sr
                qt = qtp.tile([P, O], dt.uint8, tag="qt", name=f"qt{g}")
                nc.scalar.dma_start(qt[:], qT[g * P:(g + 1) * P, :])
                w = wp.tile([P, O], dt.float16, tag=f"w{g}", name=f"w{g}")
                eng = nc.gpsimd if g % 3 == 2 else nc.vector
                eng.tensor_tensor(out=w[:], in0=qt[:],
                                  in1=srs[g // SC][:, g % SC, :],
                                  op=mybir.AluOpType.mult)
                W.append(w)
                if g == 0:
                    nc.sync.dma_start(znt[:], zneg[:])

            # --- matmul: out.T accumulated over G k-groups + zero-point block ---
            ps = [psp.tile([P, MB], dt.float32, tag=f"ps{t}", name=f"ps{t}")
                  for t in range(NOT)]
            NGB = G // XB
            for mg in range(NMG):
                at_t = atp.tile([G, MB], dt.float16, tag="at", name=f"at{mg}")
                nc.sync.dma_start(at_t[:], aT[:, mg * MB:(mg + 1) * MB])
                # zero-point block first so chains end on regular groups
                for t in range(NOT):
                    nc.tensor.matmul(ps[t][:], znt[:, t * P:(t + 1) * P],
                                     at_t[:], start=True, stop=False)
                for gb in range(NGB):
                    xg = xp.tile([P, XB, MB], dt.float16, tag="xg",
                                 name=f"xg{mg}_{gb}")
                    nc.sync.dma_start(
                        xg[:],
                        xT[gb * XB * P:(gb + 1) * XB * P,
                           mg * MB:(mg + 1) * MB].rearrange(
                               "(b p) m -> p b m", p=P))
                    if gb < NGB - 1:
                        for j in range(XB):
                            g = gb * XB + j
                            for t in range(NOT):
                                nc.tensor.matmul(ps[t][:],
                                                 W[g][:, t * P:(t + 1) * P],
                                                 xg[:, j, :],
                                                 start=False, stop=False)
                    else:
                        # last block bank-major: banks finish staggered, so
                        # PSUM copies overlap the remaining banks' matmuls
                        for t in range(NOT):
                            for j in range(XB):
                                g = gb * XB + j
                                nc.tensor.matmul(ps[t][:],
                                                 W[g][:, t * P:(t + 1) * P],
                                                 xg[:, j, :],
                                                 start=False, stop=(j == XB - 1))
                for t in range(NOT):
                    ob = obp.tile([P, MB], dt.float32, tag=f"ob{t % 2}",
                                  name=f"ob{mg}_{t}")
                    if t % 2 == 0 and mg >= 2:
                        nc.vector.tensor_copy(ob[:], ps[t][:])
                    else:
                        nc.scalar.activation(ob[:], ps[t][:],
                                             mybir.ActivationFunctionType.Copy)
                    nc.sync.dma_start(
                        outT[t * P:(t + 1) * P, mg * MB:(mg + 1) * MB], ob[:])

    _patch_nc(nc)
    return nc


def _prep_inputs(x, qweight, wscales, zeros):
    xT = x.T.astype(np.float16)

    # group sums of x for the zero-point term (fp32, exact w.r.t. reference)
    A = x.reshape(M, G, P).sum(axis=2, dtype=np.float64).astype(np.float32)
    aT = A.T.astype(np.float16)                      # [G, M]

    # unpack nibbles -> q [O_TOTAL, K] u8 (low nibble first)
    qb = qweight.astype(np.uint8)
    q = np.empty((O_TOTAL, K), np.uint8)
    q[:, 0::2] = qb & 0xF
    q[:, 1::2] = qb >> 4

    s16 = wscales.astype(np.float16)                 # [G, O_TOTAL]
    zn16 = (-zeros).astype(np.float16)               # [G, O_TOTAL]

    per_core = []
    for c in range(N_CORES):
        o0 = c * O
        qTc = np.ascontiguousarray(q[o0:o0 + O, :].T)            # [K, O]
        per_core.append({
            "xT": xT,
            "qT": qTc,
            "ssh": np.ascontiguousarray(s16[:, o0:o0 + O]),
            "zneg": np.ascontiguousarray(zn16[:, o0:o0 + O]),
            "aT": aT,
        })
    return per_core


_NC_CACHE = {}


def _get_program():
    if "nc" not in _NC_CACHE:
        _NC_CACHE["nc"] = build_program()
    return _NC_CACHE["nc"]


def run(x, qweight, wscales, zeros, trace=False, **spmd_kwargs):
    nc = _get_program()
    in_maps = _prep_inputs(x, qweight, wscales, zeros)
    res = run_bass_kernel_spmd(nc, in_maps, list(range(N_CORES)),
                               trace=trace, **spmd_kwargs)
    out = np.empty((M, O_TOTAL), np.float32)
    for c in range(N_CORES):
        out[:, c * O:(c + 1) * O] = res.results[c]["outT"].T
    return out, res


def kernel(x, qweight, wscales, zeros):
    out, _ = run(np.asarray(x), np.asarray(qweight), np.asarray(wscales),
                 np.asarray(zeros))
    return out


# revision 13
# speedup vs baseline: 1.1850x; 1.1850x over previous
"""AWQ W4A16 linear kernel for Trainium2, 8-core tensor-parallel (column-parallel).

Problem: out[m, o] = sum_k x[m, k] * w[o, k]
         w[o, k] = q[o, k] * s[g(k), o] - z[g(k), o],  GROUP_SIZE=128, g(k)=k//128
         x [4096, 8192] f32, qweight [8192, 4096] int32 (bytes, 2 nibbles each),
         wscales/zeros [64, 8192] f32, out [4096, 8192] f32.

Sharding: out_features (O) split across 8 cores, 1024 each; x replicated.

Math restructure: out = x @ (q*s).T + A @ (-z), where A[m, g] = sum_{k in g} x[m, k].
The zero-point term is exactly a rank-64 matmul, appended to the PE accumulation
chain as a K=64 contraction block; A is precomputed on the host from fp32 x.

Per-core device kernel:
  - host supplies q already unpacked to uint8 in [k, o] layout (qT), and the
    scales broadcast along partitions (srep, fp16)
  - DVE: W[g][128 k, 1024 o] = qT_tile * srep_tile  (one tensor_tensor per group)
  - PE: out.T[o-tile, m] accumulated over 64 k-groups + 1 zero-point block,
    fp16 x fp16 -> f32 PSUM, 8 PSUM banks = 8 o-tiles in flight,
    moving operand = x.T tile [128, 512]
  - DVE copies PSUM->SBUF, DMA out.T to DRAM. Host re-transposes/concats.

Note on waitfix: this walrus build caps every instruction at ONE semaphore
wait (EventSemaphore holds two). Tile emits more; we post-process the BIR
JSON, hoisting excess waits into standalone EventSemaphore instructions
inserted before the over-budget instruction on the same engine.
"""
import sys

for _p in ("/opt/trn_rl_repo", "/root/.axon_site/_ro/trn_rl_repo"):
    if _p not in sys.path:
        sys.path.append(_p)

import json
import numpy as np

import concourse.bass as bass
import concourse.mybir as mybir
import concourse.tile as tile
from concourse.bass_utils import run_bass_kernel_spmd

P = 128          # partitions / group size
M = 4096         # rows of x
K = 8192         # in features
O_TOTAL = 8192   # out features
N_CORES = 8
O = O_TOTAL // N_CORES   # 1024 per core
NOT = O // P             # 8 o-tiles per core
G = K // P               # 64 k-groups
MB = 512                 # moving free dim per matmul
NMG = M // MB            # 8 m-groups
XB = 4                   # k-groups per x-load DMA
dt = mybir.dt


def _split_excess_waits(doc):
    for f in doc.get("functions", []):
        for b in f.get("blocks", []):
            out = []
            for i in b.get("instructions", []):
                si = i.get("sync_info")
                waits = (si or {}).get("on_wait") or []
                if len(waits) > 1:
                    excess, keep = waits[:-1], waits[-1:]
                    for k in range(0, len(excess), 2):
                        out.append({
                            "name": f"{i['name']}-wsplit{k}",
                            "opcode": "EventSemaphore",
                            "engine": i["engine"],
                            "ins": [], "outs": [],
                            "debug": i.get("debug", 0),
                            "sync_info": {"on_wait": excess[k:k + 2],
                                          "on_update": []},
                        })
                    si["on_wait"] = keep
                out.append(i)
            b["instructions"] = out
    return doc


def _thin_pe_updates(doc, every=8):
    """Drop per-MATMUL semaphore increments, keeping one on every Nth matmul,
    and renumber every >=-wait on those sems to the next kept tick. Waits are
    >= so this only over-synchronizes (by at most N-1 matmuls)."""
    import bisect

    all_insts = []
    for f in doc.get("functions", []):
        for b in f.get("blocks", []):
            all_insts.extend(b.get("instructions", []))

    mm_sems = set()
    for i in all_insts:
        if i.get("opcode") == "Matmult":
            for u in (i.get("sync_info") or {}).get("on_update") or []:
                if u.get("update_mode") == "sem-inc":
                    mm_sems.add(u["id"])
    # skip sems with exact-value waits (can't renumber those safely)
    for i in all_insts:
        for w in (i.get("sync_info") or {}).get("on_wait") or []:
            if (w.get("id") in mm_sems and w.get("wait_mode") != "sem-ge-imm"
                    and w.get("wait_value", 0) != 0):
                mm_sems.discard(w["id"])

    for s in mm_sems:
        upd_sites = []
        for i in all_insts:
            for u in (i.get("sync_info") or {}).get("on_update") or []:
                if u.get("update_mode") == "sem-inc" and u.get("id") == s:
                    assert u.get("update_value", 1) == 1
                    upd_sites.append((i, u))
        T = len(upd_sites)
        if T == 0:
            continue
        keep = []
        cnt = 0
        for i, u in upd_sites:
            if i.get("opcode") != "Matmult":
                keep.append(True)
            else:
                cnt += 1
                keep.append(cnt % every == 0)
        keep[-1] = True
        pref = [0]
        for k in keep:
            pref.append(pref[-1] + (1 if k else 0))
        kept_pos = [idx + 1 for idx, k in enumerate(keep) if k]  # 1-based

        def new_wait(v):
            if v <= 0:
                return v
            p = bisect.bisect_left(kept_pos, min(v, T))
            return pref[kept_pos[p]]

        for (i, u), k in zip(upd_sites, keep):
            if not k:
                si = i["sync_info"]
                si["on_update"] = [
                    x for x in si["on_update"]
                    if not (x.get("update_mode") == "sem-inc"
                            and x.get("id") == s)]
        for i in all_insts:
            for w in (i.get("sync_info") or {}).get("on_wait") or []:
                if w.get("id") == s and w.get("wait_mode") == "sem-ge-imm":
                    w["wait_value"] = new_wait(w["wait_value"])
    return doc


def _patch_nc(nc):
    orig = nc.to_json_bytes

    def fixed():
        doc = json.loads(orig())
        doc = _thin_pe_updates(doc)
        doc = _split_excess_waits(doc)
        return json.dumps(doc).encode()

    nc.to_json_bytes = fixed


def build_program():
    nc = bass.Bass()
    xT = nc.declare_dram_parameter("xT", [K, M], dt.float16, isOutput=False)
    qT = nc.declare_dram_parameter("qT", [K, O], dt.uint8, isOutput=False)
    ssh = nc.declare_dram_parameter("ssh", [G, O], dt.float16, isOutput=False)
    zneg = nc.declare_dram_parameter("zneg", [G, O], dt.float16, isOutput=False)
    aT = nc.declare_dram_parameter("aT", [G, M], dt.float16, isOutput=False)
    outT = nc.declare_dram_parameter("outT", [O, M], dt.float32, isOutput=True)

    with tile.TileContext(nc) as tc:
        with (
            tc.tile_pool(name="cst", bufs=1) as cst,
            tc.tile_pool(name="qt", bufs=4) as qtp,
            tc.tile_pool(name="sr", bufs=3) as srp,
            tc.tile_pool(name="w", bufs=1) as wp,
            tc.tile_pool(name="xg", bufs=4) as xp,
            tc.tile_pool(name="at", bufs=2) as atp,
            tc.tile_pool(name="ob", bufs=2) as obp,
            tc.tile_pool(name="ps", bufs=1, space="PSUM") as psp,
        ):
            # --- dequant: W[g] = qT[g] * s_bcast[g], one TT op per group ---
            SC = 4  # scale-broadcast chunk (groups per DMA)
            W = []
            znt = cst.tile([G, O], dt.float16)
            srs = {}
            for g in range(G):
                if g % SC == 0:
                    sr = srp.tile([P, SC, O], dt.float16, tag="sr",
                                  name=f"sr{g // SC}")
                    nc.scalar.dma_start(
                        sr[:], ssh[g:g + SC, :].unsqueeze(0)
                        .broadcast_to([P, SC, O]))
                    srs[g // SC] = sr
                qt = qtp.tile([P, O], dt.uint8, tag="qt", name=f"qt{g}")
                nc.scalar.dma_start(qt[:], qT[g * P:(g + 1) * P, :])
                w = wp.tile([P, O], dt.float16, tag=f"w{g}", name=f"w{g}")
                eng = nc.gpsimd if g % 3 == 2 else nc.vector
                if g < 2:
                    # halves so the first o-slices land sooner at startup
                    for h in range(2):
                        sl = slice(h * O // 2, (h + 1) * O // 2)
                        eng.tensor_tensor(out=w[:, sl], in0=qt[:, sl],
                                          in1=srs[g // SC][:, g % SC, sl],
                                          op=mybir.AluOpType.mult)
                else:
                    eng.tensor_tensor(out=w[:], in0=qt[:],
                                      in1=srs[g // SC][:, g % SC, :],
                                      op=mybir.AluOpType.mult)
                W.append(w)
                if g == 0:
                    nc.sync.dma_start(znt[:], zneg[:])

            # --- matmul: out.T accumulated over G k-groups + zero-point block ---
            ps = [psp.tile([P, MB], dt.float32, tag=f"ps{t}", name=f"ps{t}")
                  for t in range(NOT)]
            NGB = G // XB
            for mg in range(NMG):
                at_t = atp.tile([G, MB], dt.float16, tag="at", name=f"at{mg}")
                nc.sync.dma_start(at_t[:], aT[:, mg * MB:(mg + 1) * MB])
                # zero-point block first so chains end on regular groups
                for t in range(NOT):
                    nc.tensor.matmul(ps[t][:], znt[:, t * P:(t + 1) * P],
                                     at_t[:], start=True, stop=False)
                for gb in range(NGB):
                    xg = xp.tile([P, XB, MB], dt.float16, tag="xg",
                                 name=f"xg{mg}_{gb}")
                    nc.sync.dma_start(
                        xg[:],
                        xT[gb * XB * P:(gb + 1) * XB * P,
                           mg * MB:(mg + 1) * MB].rearrange(
                               "(b p) m -> p b m", p=P))
                    if gb < NGB - 1:
                        for j in range(XB):
                            g = gb * XB + j
                            for t in range(NOT):
                                nc.tensor.matmul(ps[t][:],
                                                 W[g][:, t * P:(t + 1) * P],
                                                 xg[:, j, :],
                                                 start=False, stop=False)
                    else:
                        # last block bank-major: banks finish staggered, so
                        # PSUM copies overlap the remaining banks' matmuls
                        for t in range(NOT):
                            for j in range(XB):
                                g = gb * XB + j
                                nc.tensor.matmul(ps[t][:],
                                                 W[g][:, t * P:(t + 1) * P],
                                                 xg[:, j, :],
                                                 start=False, stop=(j == XB - 1))
                for t in range(NOT):
                    ob = obp.tile([P, MB], dt.float32, tag=f"ob{t % 2}",
                                  name=f"ob{mg}_{t}")
                    dst = outT[t * P:(t + 1) * P, mg * MB:(mg + 1) * MB]
                    if mg == NMG - 1 and t >= NOT - 2:
                        # stagger the final banks so the tail drains sooner
                        for q in range(4):
                            sl = slice(q * MB // 4, (q + 1) * MB // 4)
                            nc.scalar.activation(
                                ob[:, sl], ps[t][:, sl],
                                mybir.ActivationFunctionType.Copy)
                            nc.sync.dma_start(dst[:, sl], ob[:, sl])
                        continue
                    if t % 2 == 0 and mg >= 2:
                        nc.vector.tensor_copy(ob[:], ps[t][:])
                    else:
                        nc.scalar.activation(ob[:], ps[t][:],
                                             mybir.ActivationFunctionType.Copy)
                    nc.sync.dma_start(dst[:], ob[:])

    _patch_nc(nc)
    return nc


def _prep_inputs(x, qweight, wscales, zeros):
    xT = x.T.astype(np.float16)

    # group sums of x for the zero-point term (fp32, exact w.r.t. reference)
    A = x.reshape(M, G, P).sum(axis=2, dtype=np.float64).astype(np.float32)
    aT = A.T.astype(np.float16)                      # [G, M]

    # unpack nibbles -> q [O_TOTAL, K] u8 (low nibble first)
    qb = qweight.astype(np.uint8)
    q = np.empty((O_TOTAL, K), np.uint8)
    q[:, 0::2] = qb & 0xF
    q[:, 1::2] = qb >> 4

    s16 = wscales.astype(np.float16)                 # [G, O_TOTAL]
    zn16 = (-zeros).astype(np.float16)               # [G, O_TOTAL]

    per_core = []
    for c in range(N_CORES):
        o0 = c * O
        qTc = np.ascontiguousarray(q[o0:o0 + O, :].T)            # [K, O]
        per_core.append({
            "xT": xT,
            "qT": qTc,
            "ssh": np.ascontiguousarray(s16[:, o0:o0 + O]),
            "zneg": np.ascontiguousarray(zn16[:, o0:o0 + O]),
            "aT": aT,
        })
    return per_core


_NC_CACHE = {}


def _get_program():
    if "nc" not in _NC_CACHE:
        _NC_CACHE["nc"] = build_program()
    return _NC_CACHE["nc"]


def run(x, qweight, wscales, zeros, trace=False, **spmd_kwargs):
    nc = _get_program()
    in_maps = _prep_inputs(x, qweight, wscales, zeros)
    res = run_bass_kernel_spmd(nc, in_maps, list(range(N_CORES)),
                               trace=trace, **spmd_kwargs)
    out = np.empty((M, O_TOTAL), np.float32)
    for c in range(N_CORES):
        out[:, c * O:(c + 1) * O] = res.results[c]["outT"].T
    return out, res


def kernel(x, qweight, wscales, zeros):
    out, _ = run(np.asarray(x), np.asarray(qweight), np.asarray(wscales),
                 np.asarray(zeros))
    return out
